# revision 16
# baseline (speedup 1.0000x reference)
"""Trainium2 Bass kernel for the MFCA channel-attention module.

  q = x_RGB.reshape(B, C, N); k = v = x.reshape(B, C, N)
  energy    = q @ k^T                          (B, C, C)
  attention = softmax(max(energy, -1) - energy)   over last axis
  out       = delta * (attention @ v) + x

Numerically, softmax(max - energy) == softmax(-energy); the stable form is
p = exp(min_row(energy) - energy), attention = p / rowsum(p).

Sharding: data-parallel over batch B=16 across 8 NeuronCores (2 per core).

v8 (bf16 transposes straight off the load tiles; no queue HOL):
  - both q and x load as bf16 quarters (cast-DMA); the Q^T/K^T PE
    transposes read those tiles DIRECTLY, so the transpose stream depends
    only on DMA arrival -- no engine cast sits on its critical path.
    Stages are contiguous bf16; drains cast to the fp8 DoubleRow pair
    layout (same operands as before).
  - the fp8 V tiles for MM2 are cast from the x quarters LATE (after the
    quarter's transposes), so those copies never head-of-line-block the
    stage drains in the strict-FIFO ACT/DVE queues.  v7's big stall was
    exactly this: up-front cast copies waiting on future quarters while
    ready drains sat behind them.
  - PE emission is strictly arrival-ordered (v7): at a quarter boundary
    ready work (MM1(p-1), MM2 fillers) is emitted before T(p).
  - MM2 epilogue split: ACT drains u PSUM->SBUF bf16, DVE adds bf16+bf16
    at 2x rate; tail MM2 uses a 4-deep u pipeline (es banks reused);
    stores stream per (i, 1024-column) chunk.
"""

from contextlib import ExitStack

import numpy as np

import concourse.bass as bass
import concourse.tile as tile
from concourse import bacc, mybir
from concourse.bass_utils import run_bass_kernel_spmd
from concourse.masks import make_identity

N_CORES = 8
B, C, H, W = 16, 512, 64, 64
N = H * W  # 4096
BS = B // N_CORES  # batches per core

F32 = mybir.dt.float32
BF16 = mybir.dt.bfloat16
FP8 = mybir.dt.float8e4

DR = mybir.MatmulPerfMode.DoubleRow

_DONE = object()  # sentinel for generator exhaustion


def build_nc(bs=BS, c=C, n=N):
    """Build the single-core Bass program (SPMD across all cores)."""
    nc = bacc.Bacc(None, target_bir_lowering=False, debug=False)

    x_d = nc.dram_tensor("x", [bs, c, n], F32, kind="ExternalInput")
    q_d = nc.dram_tensor("x_RGB", [bs, c, n], F32, kind="ExternalInput")
    d_d = nc.dram_tensor("delta", [128, 1], F32, kind="ExternalInput")
    o_d = nc.dram_tensor("out", [bs, c, n], BF16, kind="ExternalOutput")

    nct = c // 128  # channel chunks (i-tiles / j-tiles)
    npr = n // 256  # DoubleRow n-pairs (16)
    nnb = n // 512  # n-blocks in the output matmul (8)
    PC = 1024  # load quarter columns (4 KiB descriptors)
    npc = n // PC  # quarters per chunk row (4)
    ppq = PC // 256  # pairs per quarter (4)

    with tile.TileContext(nc) as tc, ExitStack() as ctx:
        pxb = ctx.enter_context(tc.tile_pool(name="pxb", bufs=34))
        pqb = ctx.enter_context(tc.tile_pool(name="pqb", bufs=24))
        px8 = ctx.enter_context(tc.tile_pool(name="px8", bufs=2))
        pqt = ctx.enter_context(tc.tile_pool(name="pqt", bufs=4))
        pp = ctx.enter_context(tc.tile_pool(name="pp", bufs=4))
        ppt = ctx.enter_context(tc.tile_pool(name="ppt", bufs=5))
        pub = ctx.enter_context(tc.tile_pool(name="pub", bufs=6))
        pout = ctx.enter_context(tc.tile_pool(name="pout", bufs=6))
        psml = ctx.enter_context(tc.tile_pool(name="psml", bufs=8))
        pone = ctx.enter_context(tc.tile_pool(name="pone", bufs=1))
        pe_pool = ctx.enter_context(tc.tile_pool(name="pe", bufs=4, space="PSUM"))
        ptr_pool = ctx.enter_context(tc.tile_pool(name="ptr", bufs=2, space="PSUM"))
        pu_pool = ctx.enter_context(tc.tile_pool(name="pu", bufs=2, space="PSUM"))

        ident_bf = pone.tile([128, 128], BF16, name="identb", tag="identb")
        make_identity(nc, ident_bf[:])
        ident8 = pone.tile([128, 128], FP8, name="ident8", tag="ident8")
        make_identity(nc, ident8[:])
        delta_sb = pone.tile([128, 1], F32, name="delta", tag="delta")
        nc.sync.dma_start(out=delta_sb[:], in_=d_d[:])

        def emit_loads(b):
            """Load one batch in [128, PC] bf16 quarters (cast-DMA),
            quarter-major, q before x per chunk to match the transpose
            stream's consumption order."""
            xbs = [[None] * npc for _ in range(nct)]
            qbs = [[None] * npc for _ in range(nct)]
            for h in range(npc):
                cs = slice(h * PC, (h + 1) * PC)
                for k in range(nct):
                    qb = pqb.tile([128, PC], BF16)
                    nc.gpsimd.dma_start(
                        out=qb[:], in_=q_d[b, 128 * k : 128 * (k + 1), cs]
                    )
                    xb = pxb.tile([128, PC], BF16)
                    nc.gpsimd.dma_start(
                        out=xb[:], in_=x_d[b, 128 * k : 128 * (k + 1), cs]
                    )
                    qbs[k][h] = qb
                    xbs[k][h] = xb
            return xbs, qbs

        def _sl(xbs, cc, c0, w):
            """Slice [c0, c0+w) of chunk cc out of per-quarter tiles."""
            h = c0 // PC
            return xbs[cc][h][:, c0 - h * PC : c0 - h * PC + w]

        def t_stream(p, nxt, qbs, xbs):
            """Generator yielding after each bf16 PE transpose of pair p.
            Reads the raw bf16 load quarters; drains cast to the fp8
            DoubleRow pair layout."""
            h = (256 * p) // PC
            for li in range(2):
                nt = 2 * p + li
                off = 128 * nt - PC * h
                stage = ptr_pool.tile([128, 2, c], BF16, name="tstage", tag="stage")
                for cc in range(nct):
                    nc.tensor.transpose(
                        stage[:, 0, 128 * cc : 128 * (cc + 1)],
                        qbs[cc][h][:, off : off + 128],
                        ident_bf[:],
                    )
                    yield
                    nc.tensor.transpose(
                        stage[:, 1, 128 * cc : 128 * (cc + 1)],
                        xbs[cc][h][:, off : off + 128],
                        ident_bf[:],
                    )
                    yield
                if nt % 2 == 0:
                    nc.vector.tensor_copy(out=nxt[:, li, :, :], in_=stage[:])
                else:
                    nc.scalar.copy(out=nxt[:, li, :, :], in_=stage[:])

        def emit_mm1(p, es, qxt, ts):
            """4 DoubleRow matmuls (one per i-tile) for n-pair p; if ts is
            given, interleave the next pair's transposes 4-per-matmul."""
            for i in range(nct):
                nc.tensor.matmul(
                    es[i][:],
                    qxt[:, :, 0, 128 * i : 128 * (i + 1)],
                    qxt[:, :, 1, :],
                    start=(p == 0),
                    stop=(p == npr - 1),
                    perf_mode=DR,
                )
                if ts is not None:
                    for _ in range(4):
                        next(ts, None)
            if ts is not None:
                for _ in ts:
                    pass

        def emit_softmax(i, es):
            e = es[i]
            m = psml.tile([128, 1], F32)
            nc.vector.tensor_reduce(
                m[:], e[:], axis=mybir.AxisListType.X, op=mybir.AluOpType.min
            )
            p_t = pp.tile([128, c], BF16, name="p_t", tag="p_t")
            z = psml.tile([128, 1], F32)
            nc.scalar.activation(
                out=p_t[:],
                in_=e[:],
                func=mybir.ActivationFunctionType.Exp,
                bias=m[:],
                scale=-1.0,
                accum_out=z[:],
            )
            zi = psml.tile([128, 1], F32)
            nc.vector.reciprocal(zi[:], z[:])
            s = psml.tile([128, 1], F32)
            nc.vector.tensor_scalar_mul(s[:], zi[:], delta_sb[:])  # delta / Z
            # Fold delta/Z into P here so MM2 needs no per-block scaling.
            # (on ACT: DVE is the busier engine mid-kernel)
            ps = pp.tile([128, c], FP8, name="ps", tag="ps")
            nc.scalar.mul(ps[:], p_t[:], s[:])
            # P'^T via fp8 PE transposes (step-2 stage), drained to
            # [128, jt, 128] so the MM2 DoubleRow stationary is a jt-pair
            # slice.
            pstage = ptr_pool.tile(
                [128, nct, 128, 2], FP8, name="pstage", tag="stage"
            )
            for jt in range(nct):
                nc.tensor.transpose(
                    pstage[:, jt, :, 0],
                    ps[:, 128 * jt : 128 * (jt + 1)],
                    ident8[:],
                )
            pt = ppt.tile([128, nct, 128], FP8)
            nc.scalar.copy(out=pt[:], in_=pstage[:, :, :, 0])
            return pt

        def mm2_stream(b, sm, x8, xbs, upools):
            """Generator of MM2 blocks, i-major.  Per block: 2 DoubleRow
            matmuls into u (PSUM), ACT drain-cast u -> bf16 SBUF, DVE bf16
            add with the residual; stores stream per (i, 1024-col) chunk."""
            nu = len(upools)
            ublk = 0
            for i in range(nct):
                pt = sm[i]
                for nbp in range(nnb // 2):
                    ob = pout.tile([128, 1024], BF16, name=f"ob{nbp}_{i}", tag="ob")
                    for s in range(2):
                        gnb = 2 * nbp + s
                        ns = slice(512 * gnb, 512 * (gnb + 1))
                        upool, utag = upools[ublk % nu]
                        u = upool.tile([128, 512], F32, name="u", tag=utag)
                        ublk += 1
                        for jp in range(2):
                            nc.tensor.matmul(
                                u[:],
                                pt[:, 2 * jp : 2 * jp + 2, :],
                                x8[:, 2 * jp : 2 * jp + 2, ns],
                                start=(jp == 0),
                                stop=(jp == 1),
                                perf_mode=DR,
                            )
                        ub = pub.tile([128, 512], BF16, name="ub", tag="ub")
                        nc.scalar.copy(out=ub[:], in_=u[:])
                        nc.vector.tensor_add(
                            ob[:, 512 * s : 512 * (s + 1)],
                            ub[:],
                            _sl(xbs, i, 512 * gnb, 512),
                        )
                        yield
                    nc.sync.dma_start(
                        out=o_d[
                            b, 128 * i : 128 * (i + 1), 1024 * nbp : 1024 * (nbp + 1)
                        ],
                        in_=ob[:],
                    )

        def emit_x8_tile(x8, xbs, g):
            """Cast one [128, PC] x quarter-tile (global tile index g =
            4*h + k) to the fp8 V layout; engines alternate by g."""
            h, k = g // nct, g % nct
            cs = slice(h * PC, (h + 1) * PC)
            if g % 2 == 0:
                nc.scalar.copy(out=x8[:, k, cs], in_=xbs[k][h][:])
            else:
                nc.vector.tensor_copy(out=x8[:, k, cs], in_=xbs[k][h][:])

        def emit_batch_front(b, mm2, t0, warm=False):
            """Loads, transposes, energy matmuls, and softmax for one batch;
            the previous batch's MM2 blocks fill load-stall gaps.  Emission
            is strictly arrival-ordered: at a quarter boundary all ready
            work (MM1, fillers) is emitted BEFORE T(p) so the in-order PE
            queue never blocks on a load.  MM1 runs with a TWO-pair lag so
            its stage drain (DVE) is never on the PE critical path; the fp8
            V copies trail arrival by two tiles so they never block drains
            in the engine FIFOs."""
            xbs, qbs = emit_loads(b)
            x8 = px8.tile([128, nct, n], FP8, name="x8", tag="x8")
            es = [
                pe_pool.tile([128, c], F32, name=f"e{i}", tag="e") for i in range(nct)
            ]
            if warm:
                # HAM warmup: transposes don't count as PE-busy for the
                # clock gate, so issue real matmuls while waiting for the
                # first quarters (scratch PSUM, no data deps).
                wu = pu_pool.tile([128, 512], F32, name="warm", tag="u")
                for w in range(16):
                    nc.tensor.matmul(
                        wu[:, :128], ident8[:], ident8[:], start=True, stop=True
                    )
            qxts = [None] * npr
            for p in range(npr):
                with tc.tile_wait_until(t0 + 0.0027 * p):
                    qxt = pqt.tile([128, 2, 2, c], FP8, name="qxt", tag="qxt")
                    qxts[p] = qxt
                    ts = t_stream(p, qxt, qbs, xbs)
                    boundary = p % ppq == 0  # pair p starts a new quarter
                    pm = p - 2  # MM1 runs two pairs behind its transposes
                    if pm >= 0 and not boundary:
                        emit_mm1(pm, es, qxts[pm], ts)  # interleave into T(p)
                    else:
                        if pm >= 0:
                            emit_mm1(pm, es, qxts[pm], None)
                        if mm2 is not None:
                            for _ in range(2):
                                next(mm2, None)
                        elif warm and p < 2:
                            wu = pu_pool.tile([128, 512], F32, name="warm", tag="u")
                            for w in range(4):
                                nc.tensor.matmul(
                                    wu[:, :128],
                                    ident8[:],
                                    ident8[:],
                                    start=True,
                                    stop=True,
                                )
                        for _ in ts:
                            pass
                    if mm2 is not None and not boundary:
                        for _ in range(2):
                            next(mm2, None)
                    if p >= 2:
                        emit_x8_tile(x8, xbs, p - 2)
            with tc.tile_wait_until(t0 + 0.0027 * npr + 0.002):
                emit_mm1(npr - 2, es, qxts[npr - 2], None)
                emit_mm1(npr - 1, es, qxts[npr - 1], None)
                for g in (npr - 2, npr - 1):
                    emit_x8_tile(x8, xbs, g)
                if mm2 is not None:
                    for _ in mm2:
                        pass
                sm = [emit_softmax(i, es) for i in range(nct)]
            return xbs, x8, sm

        mm2 = None
        for b in range(bs):
            t0 = 0.007 + 0.047 * b
            xbs, x8, sm = emit_batch_front(b, mm2, t0, warm=(b == 0))
            tail = b == bs - 1
            upools = (
                [(pu_pool, "u"), (pu_pool, "u"), (pe_pool, "e"), (pe_pool, "e")]
                if tail
                else [(pu_pool, "u"), (pu_pool, "u")]
            )
            mm2 = mm2_stream(b, sm, x8, xbs, upools)
        blk = 0
        while True:
            with tc.tile_wait_until(0.007 + 0.047 * bs + 0.0005 * blk):
                if next(mm2, _DONE) is _DONE:
                    break
            blk += 1

    nc.compile()
    return nc


_NC_CACHE = {}


def _get_nc(key=(BS, C, N)):
    if key not in _NC_CACHE:
        _NC_CACHE[key] = build_nc(*key)
    return _NC_CACHE[key]


def _run(x, x_RGB, delta, trace=False):
    x = np.ascontiguousarray(np.asarray(x, dtype=np.float32)).reshape(B, C, N)
    xr = np.ascontiguousarray(np.asarray(x_RGB, dtype=np.float32)).reshape(B, C, N)
    d = np.asarray(delta, dtype=np.float32).reshape(-1)[0]
    d_b = np.full((128, 1), d, dtype=np.float32)

    nc = _get_nc()
    in_maps = []
    for cid in range(N_CORES):
        sl = slice(cid * BS, (cid + 1) * BS)
        in_maps.append(
            {
                "x": np.ascontiguousarray(x[sl]),
                "x_RGB": np.ascontiguousarray(xr[sl]),
                "delta": d_b,
            }
        )
    res = run_bass_kernel_spmd(nc, in_maps, core_ids=list(range(N_CORES)), trace=trace)
    out = np.concatenate(
        [np.asarray(r["out"]).astype(np.float32) for r in res.results], axis=0
    )
    return out.reshape(B, C, H, W), res


def kernel(x, x_RGB, delta):
    out, _ = _run(x, x_RGB, delta, trace=False)
    return out


# revision 18
# speedup vs baseline: 1.0788x; 1.0788x over previous
"""Trainium2 Bass kernel for the MFCA channel-attention module.

  q = x_RGB.reshape(B, C, N); k = v = x.reshape(B, C, N)
  energy    = q @ k^T                          (B, C, C)
  attention = softmax(max(energy, -1) - energy)   over last axis
  out       = delta * (attention @ v) + x

Numerically, softmax(max - energy) == softmax(-energy); the stable form is
p = exp(min_row(energy) - energy), attention = p / rowsum(p).

Sharding: data-parallel over batch B=16 across 8 NeuronCores (2 per core).

v8 (bf16 transposes straight off the load tiles; no queue HOL):
  - both q and x load as bf16 quarters (cast-DMA); the Q^T/K^T PE
    transposes read those tiles DIRECTLY, so the transpose stream depends
    only on DMA arrival -- no engine cast sits on its critical path.
    Stages are contiguous bf16; drains cast to the fp8 DoubleRow pair
    layout (same operands as before).
  - the fp8 V tiles for MM2 are cast from the x quarters LATE (after the
    quarter's transposes), so those copies never head-of-line-block the
    stage drains in the strict-FIFO ACT/DVE queues.  v7's big stall was
    exactly this: up-front cast copies waiting on future quarters while
    ready drains sat behind them.
  - PE emission is strictly arrival-ordered (v7): at a quarter boundary
    ready work (MM1(p-1), MM2 fillers) is emitted before T(p).
  - MM2 epilogue split: ACT drains u PSUM->SBUF bf16, DVE adds bf16+bf16
    at 2x rate; tail MM2 uses a 4-deep u pipeline (es banks reused);
    stores stream per (i, 1024-column) chunk.
"""

from contextlib import ExitStack

import numpy as np

import concourse.bass as bass
import concourse.tile as tile
from concourse import bacc, mybir
from concourse.bass_utils import run_bass_kernel_spmd
from concourse.masks import make_identity

N_CORES = 8
B, C, H, W = 16, 512, 64, 64
N = H * W  # 4096
BS = B // N_CORES  # batches per core

F32 = mybir.dt.float32
BF16 = mybir.dt.bfloat16
FP8 = mybir.dt.float8e4

DR = mybir.MatmulPerfMode.DoubleRow

_DONE = object()  # sentinel for generator exhaustion


def build_nc(bs=BS, c=C, n=N):
    """Build the single-core Bass program (SPMD across all cores)."""
    nc = bacc.Bacc(None, target_bir_lowering=False, debug=False)

    x_d = nc.dram_tensor("x", [bs, c, n], F32, kind="ExternalInput")
    q_d = nc.dram_tensor("x_RGB", [bs, c, n], F32, kind="ExternalInput")
    d_d = nc.dram_tensor("delta", [128, 1], F32, kind="ExternalInput")
    o_d = nc.dram_tensor("out", [bs, c, n], BF16, kind="ExternalOutput")

    nct = c // 128  # channel chunks (i-tiles / j-tiles)
    npr = n // 256  # DoubleRow n-pairs (16)
    nnb = n // 512  # n-blocks in the output matmul (8)
    PC = 1024  # load quarter columns (4 KiB descriptors)
    npc = n // PC  # quarters per chunk row (4)
    ppq = PC // 256  # pairs per quarter (4)

    with tile.TileContext(nc) as tc, ExitStack() as ctx:
        pxb = ctx.enter_context(tc.tile_pool(name="pxb", bufs=34))
        pqb = ctx.enter_context(tc.tile_pool(name="pqb", bufs=24))
        px8 = ctx.enter_context(tc.tile_pool(name="px8", bufs=2))
        pqt = ctx.enter_context(tc.tile_pool(name="pqt", bufs=4))
        pp = ctx.enter_context(tc.tile_pool(name="pp", bufs=4))
        ppt = ctx.enter_context(tc.tile_pool(name="ppt", bufs=5))
        pub = ctx.enter_context(tc.tile_pool(name="pub", bufs=6))
        pout = ctx.enter_context(tc.tile_pool(name="pout", bufs=6))
        psml = ctx.enter_context(tc.tile_pool(name="psml", bufs=8))
        pone = ctx.enter_context(tc.tile_pool(name="pone", bufs=1))
        pe_pool = ctx.enter_context(tc.tile_pool(name="pe", bufs=4, space="PSUM"))
        ptr_pool = ctx.enter_context(tc.tile_pool(name="ptr", bufs=2, space="PSUM"))
        pu_pool = ctx.enter_context(tc.tile_pool(name="pu", bufs=2, space="PSUM"))

        ident_bf = pone.tile([128, 128], BF16, name="identb", tag="identb")
        make_identity(nc, ident_bf[:])
        ident8 = pone.tile([128, 128], FP8, name="ident8", tag="ident8")
        make_identity(nc, ident8[:])
        delta_sb = pone.tile([128, 1], F32, name="delta", tag="delta")
        nc.sync.dma_start(out=delta_sb[:], in_=d_d[:])

        def emit_loads(b):
            """Load one batch in [128, PC] bf16 quarters (cast-DMA),
            quarter-major, q before x per chunk to match the transpose
            stream's consumption order."""
            xbs = [[None] * npc for _ in range(nct)]
            qbs = [[None] * npc for _ in range(nct)]
            for h in range(npc):
                cs = slice(h * PC, (h + 1) * PC)
                for k in range(nct):
                    qb = pqb.tile([128, PC], BF16)
                    nc.gpsimd.dma_start(
                        out=qb[:], in_=q_d[b, 128 * k : 128 * (k + 1), cs]
                    )
                    xb = pxb.tile([128, PC], BF16)
                    nc.gpsimd.dma_start(
                        out=xb[:], in_=x_d[b, 128 * k : 128 * (k + 1), cs]
                    )
                    qbs[k][h] = qb
                    xbs[k][h] = xb
            return xbs, qbs

        def _sl(xbs, cc, c0, w):
            """Slice [c0, c0+w) of chunk cc out of per-quarter tiles."""
            h = c0 // PC
            return xbs[cc][h][:, c0 - h * PC : c0 - h * PC + w]

        def t_stream(p, nxt, qbs, xbs):
            """Generator yielding after each bf16 PE transpose of pair p.
            Reads the raw bf16 load quarters; drains cast to the fp8
            DoubleRow pair layout."""
            h = (256 * p) // PC
            for li in range(2):
                nt = 2 * p + li
                off = 128 * nt - PC * h
                stage = ptr_pool.tile([128, 2, c], BF16, name="tstage", tag="stage")
                for cc in range(nct):
                    nc.tensor.transpose(
                        stage[:, 0, 128 * cc : 128 * (cc + 1)],
                        qbs[cc][h][:, off : off + 128],
                        ident_bf[:],
                    )
                    yield
                    nc.tensor.transpose(
                        stage[:, 1, 128 * cc : 128 * (cc + 1)],
                        xbs[cc][h][:, off : off + 128],
                        ident_bf[:],
                    )
                    yield
                if nt % 8 < 5:  # 20 DVE / 12 ACT per batch (ACT is slower)
                    nc.vector.tensor_copy(out=nxt[:, li, :, :], in_=stage[:])
                else:
                    nc.scalar.copy(out=nxt[:, li, :, :], in_=stage[:])

        def emit_mm1(p, es, qxt, ts):
            """4 DoubleRow matmuls (one per i-tile) for n-pair p; if ts is
            given, interleave the next pair's transposes 4-per-matmul."""
            for i in range(nct):
                nc.tensor.matmul(
                    es[i][:],
                    qxt[:, :, 0, 128 * i : 128 * (i + 1)],
                    qxt[:, :, 1, :],
                    start=(p == 0),
                    stop=(p == npr - 1),
                    perf_mode=DR,
                )
                if ts is not None:
                    for _ in range(4):
                        next(ts, None)
            if ts is not None:
                for _ in ts:
                    pass

        def emit_softmax(i, es):
            e = es[i]
            m = psml.tile([128, 1], F32)
            nc.vector.tensor_reduce(
                m[:], e[:], axis=mybir.AxisListType.X, op=mybir.AluOpType.min
            )
            p_t = pp.tile([128, c], BF16, name="p_t", tag="p_t")
            z = psml.tile([128, 1], F32)
            nc.scalar.activation(
                out=p_t[:],
                in_=e[:],
                func=mybir.ActivationFunctionType.Exp,
                bias=m[:],
                scale=-1.0,
                accum_out=z[:],
            )
            zi = psml.tile([128, 1], F32)
            nc.vector.reciprocal(zi[:], z[:])
            s = psml.tile([128, 1], F32)
            nc.vector.tensor_scalar_mul(s[:], zi[:], delta_sb[:])  # delta / Z
            # Fold delta/Z into P here so MM2 needs no per-block scaling.
            # (on ACT: DVE is the busier engine mid-kernel)
            ps = pp.tile([128, c], FP8, name="ps", tag="ps")
            nc.scalar.mul(ps[:], p_t[:], s[:])
            # P'^T via fp8 PE transposes (step-2 stage), drained to
            # [128, jt, 128] so the MM2 DoubleRow stationary is a jt-pair
            # slice.
            pstage = ptr_pool.tile(
                [128, nct, 128, 2], FP8, name="pstage", tag="stage"
            )
            for jt in range(nct):
                nc.tensor.transpose(
                    pstage[:, jt, :, 0],
                    ps[:, 128 * jt : 128 * (jt + 1)],
                    ident8[:],
                )
            pt = ppt.tile([128, nct, 128], FP8)
            nc.scalar.copy(out=pt[:], in_=pstage[:, :, :, 0])
            return pt

        def mm2_stream(b, sm, x8, xbs, upools):
            """Generator of MM2 blocks, i-major.  Per block: 2 DoubleRow
            matmuls into u (PSUM), ACT drain-cast u -> bf16 SBUF, DVE bf16
            add with the residual; stores stream per (i, 1024-col) chunk."""
            nu = len(upools)
            ublk = 0
            for i in range(nct):
                pt = sm[i]
                for nbp in range(nnb // 2):
                    ob = pout.tile([128, 1024], BF16, name=f"ob{nbp}_{i}", tag="ob")
                    for s in range(2):
                        gnb = 2 * nbp + s
                        ns = slice(512 * gnb, 512 * (gnb + 1))
                        upool, utag = upools[ublk % nu]
                        u = upool.tile([128, 512], F32, name="u", tag=utag)
                        ublk += 1
                        for jp in range(2):
                            nc.tensor.matmul(
                                u[:],
                                pt[:, 2 * jp : 2 * jp + 2, :],
                                x8[:, 2 * jp : 2 * jp + 2, ns],
                                start=(jp == 0),
                                stop=(jp == 1),
                                perf_mode=DR,
                            )
                        ub = pub.tile([128, 512], BF16, name="ub", tag="ub")
                        nc.scalar.copy(out=ub[:], in_=u[:])
                        nc.vector.tensor_add(
                            ob[:, 512 * s : 512 * (s + 1)],
                            ub[:],
                            _sl(xbs, i, 512 * gnb, 512),
                        )
                        yield
                    nc.sync.dma_start(
                        out=o_d[
                            b, 128 * i : 128 * (i + 1), 1024 * nbp : 1024 * (nbp + 1)
                        ],
                        in_=ob[:],
                    )

        def emit_x8_tile(x8, xbs, g):
            """Cast one [128, PC] x quarter-tile (global tile index g =
            4*h + k) to the fp8 V layout; engines alternate by g."""
            h, k = g // nct, g % nct
            cs = slice(h * PC, (h + 1) * PC)
            nc.vector.tensor_copy(out=x8[:, k, cs], in_=xbs[k][h][:])

        def emit_batch_front(b, mm2, t0, warm=False):
            """Loads, transposes, energy matmuls, and softmax for one batch;
            the previous batch's MM2 blocks fill load-stall gaps.  Emission
            is strictly arrival-ordered: at a quarter boundary all ready
            work (MM1, fillers) is emitted BEFORE T(p) so the in-order PE
            queue never blocks on a load.  MM1 runs with a TWO-pair lag so
            its stage drain (DVE) is never on the PE critical path; the fp8
            V copies trail arrival by two tiles so they never block drains
            in the engine FIFOs."""
            xbs, qbs = emit_loads(b)
            x8 = px8.tile([128, nct, n], FP8, name="x8", tag="x8")
            es = [
                pe_pool.tile([128, c], F32, name=f"e{i}", tag="e") for i in range(nct)
            ]
            if warm:
                # HAM warmup: transposes don't count as PE-busy for the
                # clock gate, so issue real matmuls while waiting for the
                # first quarters (scratch PSUM, no data deps).
                wu = pu_pool.tile([128, 512], F32, name="warm", tag="u")
                for w in range(16):
                    nc.tensor.matmul(
                        wu[:, :128], ident8[:], ident8[:], start=True, stop=True
                    )
            qxts = [None] * npr
            for p in range(npr):
                with tc.tile_wait_until(t0 + 0.0027 * p):
                    qxt = pqt.tile([128, 2, 2, c], FP8, name="qxt", tag="qxt")
                    qxts[p] = qxt
                    ts = t_stream(p, qxt, qbs, xbs)
                    boundary = p % ppq == 0  # pair p starts a new quarter
                    pm = p - 2  # MM1 runs two pairs behind its transposes
                    if pm >= 0 and not boundary:
                        emit_mm1(pm, es, qxts[pm], ts)  # interleave into T(p)
                    else:
                        if pm >= 0:
                            emit_mm1(pm, es, qxts[pm], None)
                        if mm2 is not None:
                            for _ in range(2):
                                next(mm2, None)
                        elif warm and p < 2:
                            wu = pu_pool.tile([128, 512], F32, name="warm", tag="u")
                            for w in range(4):
                                nc.tensor.matmul(
                                    wu[:, :128],
                                    ident8[:],
                                    ident8[:],
                                    start=True,
                                    stop=True,
                                )
                        for _ in ts:
                            pass
                    if mm2 is not None and not boundary:
                        for _ in range(2):
                            next(mm2, None)
                    if p >= 2:
                        emit_x8_tile(x8, xbs, p - 2)
            with tc.tile_wait_until(t0 + 0.0027 * npr + 0.002):
                emit_mm1(npr - 2, es, qxts[npr - 2], None)
                emit_mm1(npr - 1, es, qxts[npr - 1], None)
                for g in (npr - 2, npr - 1):
                    emit_x8_tile(x8, xbs, g)
                if mm2 is not None:
                    for _ in mm2:
                        pass
                sm = [emit_softmax(i, es) for i in range(nct)]
            return xbs, x8, sm

        mm2 = None
        for b in range(bs):
            t0 = 0.007 + 0.047 * b
            xbs, x8, sm = emit_batch_front(b, mm2, t0, warm=(b == 0))
            tail = b == bs - 1
            upools = (
                [(pu_pool, "u"), (pu_pool, "u"), (pe_pool, "e"), (pe_pool, "e")]
                if tail
                else [(pu_pool, "u"), (pu_pool, "u")]
            )
            mm2 = mm2_stream(b, sm, x8, xbs, upools)
        blk = 0
        while True:
            with tc.tile_wait_until(0.007 + 0.047 * bs + 0.0005 * blk):
                if next(mm2, _DONE) is _DONE:
                    break
            blk += 1

    nc.compile()
    return nc


_NC_CACHE = {}


def _get_nc(key=(BS, C, N)):
    if key not in _NC_CACHE:
        _NC_CACHE[key] = build_nc(*key)
    return _NC_CACHE[key]


def _run(x, x_RGB, delta, trace=False):
    x = np.ascontiguousarray(np.asarray(x, dtype=np.float32)).reshape(B, C, N)
    xr = np.ascontiguousarray(np.asarray(x_RGB, dtype=np.float32)).reshape(B, C, N)
    d = np.asarray(delta, dtype=np.float32).reshape(-1)[0]
    d_b = np.full((128, 1), d, dtype=np.float32)

    nc = _get_nc()
    in_maps = []
    for cid in range(N_CORES):
        sl = slice(cid * BS, (cid + 1) * BS)
        in_maps.append(
            {
                "x": np.ascontiguousarray(x[sl]),
                "x_RGB": np.ascontiguousarray(xr[sl]),
                "delta": d_b,
            }
        )
    res = run_bass_kernel_spmd(nc, in_maps, core_ids=list(range(N_CORES)), trace=trace)
    out = np.concatenate(
        [np.asarray(r["out"]).astype(np.float32) for r in res.results], axis=0
    )
    return out.reshape(B, C, H, W), res


def kernel(x, x_RGB, delta):
    out, _ = _run(x, x_RGB, delta, trace=False)
    return out
